# revision 1
# baseline (speedup 1.0000x reference)
"""GATv2 (2-layer) + BatchNorm + skip + ELU + graph readout on 8 Trainium2
NeuronCores via Bass/Tile.

Sharding: graph-parallel. 16 graphs -> 8 cores, 2 graphs per core, each graph
padded to a fixed 128-aligned slot so the SPMD program is identical across
cores.  Edges are sharded by destination node (the core that owns dst's graph
owns the edge), sorted by destination, and grouped into 64-node destination
windows so the scatter-softmax segment reduction becomes a sequence of
one-hot matmuls accumulating in PSUM.  Source-node features are fetched from
an AllGathered (replicated) DRAM table with batched indirect-DMA gathers.
Weights are replicated.

Algebraic reformulation (exact, up to float rounding):
  GATv2 logit  l_e = sum_c att_c * leaky_relu(s_c),  s = xl[src]+xr[dst]+e.
  Fold |att_c| into Wl/Wr/We columns (tables hold s~_c = s_c*|att_c|), then
  leaky_relu(u,0.2) = 0.6u + 0.4|u| and positive homogeneity give
    l = 0.6*sum_c att_c s_c + 0.4*(sum_{c:att>0}|s~_c| - sum_{c:att<0}|s~_c|).
  The linear part is a per-node/per-edge-attr dot product precomputed as two
  extra table columns; the abs part is abs-reduces over sign-grouped channel
  ranges (channels are permuted host-side; BatchNorm is invariant to the
  per-channel |att| scaling and the permutation is undone on the host).
  Softmax is computed without the segment-max shift (logits are O(+-5), exact
  in fp32).
"""

import math
import numpy as np

N_NODES = 50000
N_EDGES = 800000
N_GRAPHS = 16
F_IN = 128
E_DIM = 16
HID = 32
HEADS = 2
EPS_BN = 1e-5
N_CORES = 8

WIN = 64          # scatter window (nodes per one-hot matmul column block)
BLK = 32          # edge tiles (128 edges each) per gather block
EGRP = 4          # edge tiles per e-matmul PSUM group
NEG_INF = -1.0e30


# ----------------------------------------------------------------------------
# host-side preparation (numpy only)
# ----------------------------------------------------------------------------

def _fold_layer(Wl, bl, Wr, br, We, att, bias, heads, hid):
    """Fold |att| + sign-permutation + 0.6/0.4 split into extended weights.

    Returns dict with Wl_ext [fin, D+H], bl_ext [D+H], (same for r),
    We_ext17 [e_dim+1, D+H] (last row = bl_ext+br_ext), bias_ext [D],
    perm [D], k_pos [H].
    """
    D = heads * hid
    a = att.reshape(heads, hid)
    perm = np.zeros(D, np.int64)
    k_pos = []
    for h in range(heads):
        ah = a[h]
        pos = np.where(ah >= 0)[0]
        neg = np.where(ah < 0)[0]
        order = np.concatenate([pos, neg])
        perm[h * hid:(h + 1) * hid] = h * hid + order
        k_pos.append(len(pos))
    colscale = np.abs(att.reshape(-1)[perm])          # [D]
    assert np.all(colscale > 0), "att has exact zeros; folding invalid"

    def ext(W, b):
        Ws = W[:, perm] * colscale[None, :]
        lin = np.stack([1.5 * (W[:, h * hid:(h + 1) * hid] @ a[h])
                        for h in range(heads)], axis=1)   # [fin, H]
        b_lin = np.array([1.5 * float(b[h * hid:(h + 1) * hid] @ a[h])
                          for h in range(heads)])
        return (np.concatenate([Ws, lin], 1).astype(np.float32),
                np.concatenate([b[perm] * colscale, b_lin]).astype(np.float32))

    Wl_ext, bl_ext = ext(Wl, bl)
    Wr_ext, br_ext = ext(Wr, br)
    We_s = We[:, perm] * colscale[None, :]
    We_lin = np.stack([1.5 * (We[:, h * hid:(h + 1) * hid] @ a[h])
                       for h in range(heads)], axis=1)
    We_ext = np.concatenate([We_s, We_lin], 1)
    # biases live in the xl/xr tables (rank-1 matmul); e has none -> zero row
    We_ext17 = np.concatenate([We_ext, np.zeros((1, We_ext.shape[1]))], 0)
    return dict(Wl_ext=Wl_ext, bl_ext=bl_ext, Wr_ext=Wr_ext, br_ext=br_ext,
                We_ext17=We_ext17.astype(np.float32),
                bias_ext=(bias[perm] * colscale).astype(np.float32),
                perm=perm, k_pos=k_pos, colscale=colscale)


def host_prep(inputs, cfg):
    nc_ = cfg["n_cores"]
    N, E, G = cfg["n_nodes"], cfg["n_edges"], cfg["n_graphs"]
    fin, edim, hid, heads = cfg["f_in"], cfg["e_dim"], cfg["hid"], cfg["heads"]
    D1 = heads * hid
    DE1 = D1 + heads
    D2 = hid
    DE2 = D2 + 1

    x = np.asarray(inputs["x"], np.float32)
    ei = np.asarray(inputs["edge_index"], np.int64)
    ea = np.asarray(inputs["edge_attr"], np.float32)
    batch = np.asarray(inputs["batch"], np.int64)

    # graph boundaries (batch is sorted)
    gs = np.searchsorted(batch, np.arange(G), side="left")
    ge = np.searchsorted(batch, np.arange(G), side="right")
    sizes = (ge - gs).astype(np.int64)
    gpg = cfg["graphs_per_core"]
    assert G == nc_ * gpg
    GP = max(128, int(math.ceil(max(1, sizes.max()) / 128.0)) * 128)
    SHP = gpg * GP
    NT = SHP // 128          # node tiles per core
    NWIN = SHP // WIN        # scatter windows per core

    # node global id -> (core, local-slot id)
    slot_of_graph = np.arange(G) % gpg
    core_of_graph = np.arange(G) // gpg
    node_core = core_of_graph[batch]
    node_local = slot_of_graph[batch] * GP + (np.arange(N) - gs[batch])
    node_pid = node_core * SHP + node_local      # padded global id

    # ---------------- edges ----------------
    src, dst = ei[0], ei[1]
    e_core = node_core[dst]
    dl_all = node_local[dst]

    # split the global (padded) node table into two halves so gather indices
    # fit int16 (dma_gather restriction); edges are grouped per
    # (dst-window, src-half) and processed in two passes (lo then hi).
    HALF = (nc_ * SHP + 1) // 2
    src_half = (node_pid[src] >= HALF).astype(np.int64)

    per_core = []
    Tw = [1, 1]
    for c in range(nc_):
        em = np.where(e_core == c)[0]
        dl = dl_all[em]
        hf = src_half[em]
        order = np.lexsort((dl, hf, dl // WIN))
        em = em[order]
        dl = dl[order]
        hf = hf[order]
        w = dl // WIN
        key = w * 2 + hf
        cnt = np.bincount(key, minlength=NWIN * 2)
        for s_ in range(2):
            cm = cnt[s_::2].max() if len(em) else 0
            Tw[s_] = max(Tw[s_], int(np.ceil(cm / 128.0)), 1)
        per_core.append((em, dl, w, hf, cnt))

    passes = []
    for s_ in range(2):
        T_s = NWIN * Tw[s_]
        passes.append(dict(
            T=T_s,
            src16=np.zeros((nc_, 128, T_s * 8), np.int16),
            dst16=np.zeros((nc_, 128, T_s * 8), np.int16),
            dstrel=np.full((nc_, 128, T_s), 999.0, np.float32),
            eaT=np.zeros((nc_, edim + 1, T_s * 128), np.float32),
        ))

    def pack_sd(P_):
        # [src 8*BLK | dst 8*BLK | drel-as-2xi16 2*BLK] per block (18*BLK stride)
        T_s = P_["T"]
        nblk = int(math.ceil(T_s / BLK))
        out = np.zeros((nc_, 128, nblk * 18 * BLK), np.int16)
        dr16 = P_["dstrel"].view(np.int16)    # [nc, 128, T*2]
        for b in range(nblk):
            t0 = b * BLK
            bt = min(BLK, T_s - t0)
            o = b * 18 * BLK
            out[:, :, o:o + bt * 8] = P_["src16"][:, :, t0 * 8:(t0 + bt) * 8]
            out[:, :, o + 8 * BLK:o + 8 * BLK + bt * 8] = \
                P_["dst16"][:, :, t0 * 8:(t0 + bt) * 8]
            out[:, :, o + 16 * BLK:o + 16 * BLK + bt * 2] = \
                dr16[:, :, t0 * 2:(t0 + bt) * 2]
        return out

    def wrap16(dstarr, c, slot, vals):
        # dma_gather index layout: idx k -> [k%16 (+16r), k//16], int16
        col = slot // 16
        row = slot % 16
        for r in range(8):
            dstarr[c, row + 16 * r, col] = vals

    for c in range(nc_):
        em, dl, w, hf, cnt = per_core[c]
        if len(em) == 0:
            continue
        kstart = np.concatenate([[0], np.cumsum(cnt)])[:-1]
        key = w * 2 + hf
        rank = np.arange(len(em)) - kstart[key]
        for s_ in range(2):
            P_ = passes[s_]
            m = hf == s_
            slot = w[m] * (Tw[s_] * 128) + rank[m]
            tile_i = slot // 128
            lane = slot % 128
            wrap16(P_["src16"], c, slot,
                   (node_pid[src[em[m]]] - s_ * HALF).astype(np.int16))
            wrap16(P_["dst16"], c, slot, dl[m].astype(np.int16))
            P_["dstrel"][c, lane, tile_i] = (dl[m] - w[m] * WIN).astype(np.float32)
            P_["eaT"][c, :edim, slot] = ea[em[m]].astype(np.float32)
            P_["eaT"][c, edim, slot] = 1.0

    # ---------------- phase0: raw edge_attr sums ----------------
    Eshard = (E + nc_ - 1) // nc_
    flat_per = Eshard * edim
    n_ph0 = int(math.ceil(flat_per / (128 * 128)))
    ea_ph0 = np.zeros((nc_, n_ph0, 128, 128), np.float32)
    eaflat = ea.reshape(-1)
    for c in range(nc_):
        seg = eaflat[c * flat_per:(c + 1) * flat_per]
        buf = np.zeros(n_ph0 * 128 * 128, np.float32)
        buf[:len(seg)] = seg
        ea_ph0[c] = buf.reshape(n_ph0, 128, 128)
    # groups-per-partition g in [0,8): flat m = g*edim + ch
    n_grp = (128 // edim)

    # ---------------- weights ----------------
    L1 = _fold_layer(inputs["conv1_Wl"], inputs["conv1_bl"], inputs["conv1_Wr"],
                     inputs["conv1_br"], inputs["conv1_We"], inputs["conv1_att"],
                     inputs["conv1_bias"], heads, hid)
    perm1 = L1["perm"]
    L2 = _fold_layer(np.asarray(inputs["conv2_Wl"])[perm1], inputs["conv2_bl"],
                     np.asarray(inputs["conv2_Wr"])[perm1], inputs["conv2_br"],
                     inputs["conv2_We"], inputs["conv2_att"],
                     inputs["conv2_bias"], 1, hid)
    perm2 = L2["perm"]

    skip1_W = np.asarray(inputs["skip1_W"], np.float32)[:, perm1]
    skip1_b = np.asarray(inputs["skip1_b"], np.float32)[perm1]
    skip2_W = np.asarray(inputs["skip2_W"], np.float32)[perm1][:, perm2]
    skip2_b = np.asarray(inputs["skip2_b"], np.float32)[perm2]

    W1cat = np.concatenate([L1["Wl_ext"], L1["Wr_ext"], skip1_W], 1)  # [fin, 2*DE1+D1]
    b1cat = np.concatenate([L1["bl_ext"], L1["br_ext"], skip1_b])[None, :]
    W2cat = np.concatenate([L2["Wl_ext"], L2["Wr_ext"], skip2_W], 1)  # [D1, 2*DE2+D2]
    b2cat = np.concatenate([L2["bl_ext"], L2["br_ext"], skip2_b])[None, :]

    # col 2: per-channel eps scaled by colscale^2 (BN eps does not commute
    # with the |att| column scaling otherwise)
    bn1 = np.stack([np.asarray(inputs["bn1_g"], np.float32)[perm1],
                    np.asarray(inputs["bn1_b"], np.float32)[perm1],
                    (EPS_BN * L1["colscale"] ** 2).astype(np.float32)], 1)
    bn2 = np.stack([np.asarray(inputs["bn2_g"], np.float32)[perm2],
                    np.asarray(inputs["bn2_b"], np.float32)[perm2],
                    (EPS_BN * L2["colscale"] ** 2).astype(np.float32)], 1)

    # ---------------- per-core node features + masks ----------------
    xT = np.zeros((nc_, fin, SHP), np.float32)
    maskB = np.zeros((nc_, 128, NT), np.float32)   # 1 = real node
    maskA = np.full((nc_, 128, NT), NEG_INF, np.float32)  # 0 = real node
    inv_cnt = np.zeros((nc_, 1, gpg), np.float32)
    for c in range(nc_):
        for s in range(gpg):
            g = c * gpg + s
            n = int(sizes[g])
            if n == 0:
                continue
            xT[c, :, s * GP:s * GP + n] = x[gs[g]:ge[g]].T
            loc = s * GP + np.arange(n)
            maskB[c, loc % 128, loc // 128] = 1.0
            maskA[c, loc % 128, loc // 128] = 0.0
            inv_cnt[c, 0, s] = 1.0 / n

    sdA = pack_sd(passes[0])
    sdB = pack_sd(passes[1])
    in_maps = []
    for c in range(nc_):
        in_maps.append({
            "xT": xT[c],
            "sdA": sdA[c], "drelA": passes[0]["dstrel"][c],
            "eaTA": passes[0]["eaT"][c],
            "sdB": sdB[c], "drelB": passes[1]["dstrel"][c],
            "eaTB": passes[1]["eaT"][c],
            "ea_ph0": ea_ph0[c].reshape(n_ph0 * 128, 128),
            "W1cat": W1cat, "b1cat": b1cat, "We1": L1["We_ext17"],
            "W2cat": W2cat, "b2cat": b2cat, "We2": L2["We_ext17"],
            "bias1": L1["bias_ext"][None, :], "bias2": L2["bias_ext"][None, :],
            "bn1": bn1, "bn2": bn2,
            "maskB": maskB[c], "maskA": maskA[c], "inv_cnt": inv_cnt[c],
        })

    meta = dict(GP=GP, SHP=SHP, NT=NT, NWIN=NWIN, TwA=Tw[0], TwB=Tw[1],
                TA=passes[0]["T"], TB=passes[1]["T"], HALF=HALF,
                n_ph0=n_ph0, n_grp=n_grp, D1=D1, DE1=DE1, D2=D2, DE2=DE2,
                k_pos1=L1["k_pos"], k_pos2=L2["k_pos"],
                perm2=perm2, sizes=sizes, gpg=gpg,
                heads=heads, hid=hid, fin=fin, edim=edim,
                E=E, N=N, n_cores=nc_)
    return in_maps, meta


# ----------------------------------------------------------------------------
# device program
# ----------------------------------------------------------------------------

def build_program(meta, dbg=False, no_cc=False, skip=()):
    import concourse.bass as bass
    import concourse.tile as tile
    import concourse.mybir as mybir
    from concourse import bacc
    from concourse.masks import make_identity
    from contextlib import ExitStack

    f32 = mybir.dt.float32
    i32 = mybir.dt.int32
    AF = mybir.ActivationFunctionType
    OP = mybir.AluOpType

    ncores = meta["n_cores"]
    GP, SHP, NT, NWIN = (meta[k] for k in ("GP", "SHP", "NT", "NWIN"))
    TwA, TwB, TA, TB = meta["TwA"], meta["TwB"], meta["TA"], meta["TB"]
    HALF = meta["HALF"]
    n_ph0, n_grp = meta["n_ph0"], meta["n_grp"]
    DEp1, DEp2 = 128, 64
    D1, DE1, D2, DE2 = meta["D1"], meta["DE1"], meta["D2"], meta["DE2"]
    heads, hid, fin, edim = meta["heads"], meta["hid"], meta["fin"], meta["edim"]
    gpg = meta["gpg"]
    E, N = meta["E"], meta["N"]

    nc = bacc.Bacc("TRN2", target_bir_lowering=False, debug=False,
                   num_devices=ncores)
    rg = [list(range(ncores))]

    def din(name, shape, dtype=f32):
        return nc.dram_tensor(name, list(shape), dtype, kind="ExternalInput").ap()

    def dout(name, shape, dtype=f32):
        return nc.dram_tensor(name, list(shape), dtype, kind="ExternalOutput").ap()

    i16 = mybir.dt.int16
    nblkA = int(math.ceil(TA / BLK))
    nblkB = int(math.ceil(TB / BLK))
    xT_d = din("xT", [fin, SHP])
    sdA_d = din("sdA", [128, nblkA * 18 * BLK], i16)
    drelA_d = din("drelA", [128, TA])
    eaTA_d = din("eaTA", [edim + 1, TA * 128])
    sdB_d = din("sdB", [128, nblkB * 18 * BLK], i16)
    drelB_d = din("drelB", [128, TB])
    eaTB_d = din("eaTB", [edim + 1, TB * 128])
    ph0_d = din("ea_ph0", [n_ph0 * 128, 128])
    W1cat_d = din("W1cat", [fin, 2 * DE1 + D1])
    b1cat_d = din("b1cat", [1, 2 * DE1 + D1])
    We1_d = din("We1", [edim + 1, DE1])
    W2cat_d = din("W2cat", [D1, 2 * DE2 + D2])
    b2cat_d = din("b2cat", [1, 2 * DE2 + D2])
    We2_d = din("We2", [edim + 1, DE2])
    bias1_d = din("bias1", [1, D1])
    bias2_d = din("bias2", [1, D2])
    bn1_d = din("bn1", [D1, 3])
    bn2_d = din("bn2", [D2, 3])
    maskB_d = din("maskB", [128, NT])
    maskA_d = din("maskA", [128, NT])
    invc_d = din("inv_cnt", [1, gpg])

    gsum_d = dout("gsum", [hid, gpg])
    gmax_d = dout("gmax", [hid, gpg])
    NT_ = SHP // 128
    if dbg:
        dbg_xl1 = dout("dbg_xl1", [SHP, 128])
        dbg_xr1 = dout("dbg_xr1", [SHP, 128])
        dbg_acc1 = dout("dbg_acc1", [128, NT_ * DE1])
        dbg_y1 = dout("dbg_y1", [128, NT_ * D1])
        dbg_h1 = dout("dbg_h1", [128, NT_ * D1])
        dbg_c1 = dout("dbg_c1", [1, DE1])
        dbg_st1 = dout("dbg_st1", [D1, 2])
        dbg_acc2 = dout("dbg_acc2", [128, NT_ * DE2])
        dbg_h2 = dout("dbg_h2", [128, NT_ * D2])

    with tile.TileContext(nc) as tc, ExitStack() as ctx:
        singles = ctx.enter_context(tc.tile_pool(name="singles", bufs=1))
        dense = ctx.enter_context(tc.tile_pool(name="dense", bufs=3))
        # ONE shared psum tag (2 banks x 2 bufs) + window psum (1 bank x 4)
        epsum = ctx.enter_context(tc.tile_pool(name="epsum", bufs=4, space="PSUM"))
        wpsum = ctx.enter_context(tc.tile_pool(name="wpsum", bufs=4, space="PSUM"))
        dram = ctx.enter_context(tc.tile_pool(name="dram", bufs=1, space="DRAM"))
        big = ctx.enter_context(tc.tile_pool(name="big", bufs=1))
        blkp = ctx.enter_context(tc.tile_pool(name="blkp", bufs=2))
        small = ctx.enter_context(tc.tile_pool(name="small", bufs=2))

        def psum_tile(shape):
            return epsum.tile(shape, f32, space="PSUM", tag="eps", name="eps")

        # ---- constants ----
        ident = singles.tile([128, 128], f32)
        make_identity(nc, ident[:])
        iota_i = singles.tile([128, WIN], i32)
        nc.gpsimd.iota(iota_i[:], pattern=[[1, WIN]], base=0, channel_multiplier=0)
        iota_f = singles.tile([128, WIN], f32)
        nc.vector.tensor_copy(iota_f[:], iota_i[:])
        ones1 = singles.tile([1, 128], f32)
        nc.vector.memset(ones1[:], 1.0)
        onesK = singles.tile([128, 1], f32)
        nc.vector.memset(onesK[:], 1.0)

        W1cat = singles.tile([fin, 2 * DE1 + D1], f32)
        nc.sync.dma_start(out=W1cat[:], in_=W1cat_d[:, :])
        b1cat = singles.tile([1, 2 * DE1 + D1], f32)
        nc.sync.dma_start(out=b1cat[:], in_=b1cat_d[:, :])
        We1 = singles.tile([edim + 1, DE1], f32)
        nc.sync.dma_start(out=We1[:], in_=We1_d[:, :])
        W2cat = singles.tile([D1, 2 * DE2 + D2], f32)
        nc.sync.dma_start(out=W2cat[:], in_=W2cat_d[:, :])
        b2cat = singles.tile([1, 2 * DE2 + D2], f32)
        nc.sync.dma_start(out=b2cat[:], in_=b2cat_d[:, :])
        We2 = singles.tile([edim + 1, DE2], f32)
        nc.sync.dma_start(out=We2[:], in_=We2_d[:, :])
        bn1sb = singles.tile([D1, 3], f32)
        nc.sync.dma_start(out=bn1sb[:], in_=bn1_d[:, :])
        bn2sb = singles.tile([D2, 3], f32)
        nc.sync.dma_start(out=bn2sb[:], in_=bn2_d[:, :])
        maskB = singles.tile([128, NT], f32)
        nc.sync.dma_start(out=maskB[:], in_=maskB_d[:, :])
        maskA = singles.tile([128, NT], f32)
        nc.sync.dma_start(out=maskA[:], in_=maskA_d[:, :])

        def bcast_row(dr_ap, n_cols, n_part=128):
            """DMA-broadcast a DRAM row [1, n_cols] to [n_part, n_cols]."""
            t = singles.tile([n_part, n_cols], f32, tag=f"bc{dr_ap.name}{n_cols}")
            src = bass.AP(tensor=dr_ap.tensor, offset=dr_ap.offset,
                          ap=[[0, n_part]] + [list(p) for p in dr_ap.ap[1:]])
            nc.sync.dma_start(out=t[:], in_=src)
            return t

        bias1b = bcast_row(bias1_d[:, :], D1)
        bias2b = bcast_row(bias2_d[:, :], D2)

        # ---- DRAM scratch ----
        XL1_shard = dram.tile([SHP, DEp1], f32)
        XR1 = dram.tile([SHP, DEp1], f32)
        XL1_full = dram.tile([ncores * SHP, DEp1], f32)
        XL2_shard = dram.tile([SHP, DEp2], f32)
        XR2 = dram.tile([SHP, DEp2], f32)
        XL2_full = dram.tile([ncores * SHP, DEp2], f32)
        sae_l = dram.tile([128, 1], f32)
        sae_g = dram.tile([128, 1], f32)
        st1_l = dram.tile([D1 * 2, 1], f32)
        st1_g = dram.tile([D1 * 2, 1], f32)
        st2_l = dram.tile([D2 * 2, 1], f32)
        st2_g = dram.tile([D2 * 2, 1], f32)
        c1row = dram.tile([1, DE1], f32)
        c2row = dram.tile([1, DE2], f32)
        gs1row = dram.tile([1, D1], f32)
        sh1row = dram.tile([1, D1], f32)
        gs2row = dram.tile([1, D2], f32)
        sh2row = dram.tile([1, D2], f32)

        # zero the padded table columns (gathers read full padded rows)
        zpad = singles.tile([128, DEp1 - DE1], f32)
        nc.vector.memset(zpad[:], 0.0)
        for tbl, DE_, DEp_ in ((XL1_shard, DE1, DEp1), (XR1, DE1, DEp1),
                               (XL2_shard, DE2, DEp2), (XR2, DE2, DEp2)):
            zsrc = bass.AP(tensor=zpad.tensor, offset=zpad[:].offset,
                           ap=[list(zpad[:].ap[0]), [0, NT], [1, DEp_ - DE_]])
            zdst = bass.AP(tensor=tbl.tensor, offset=tbl[:].offset + DE_,
                           ap=[[DEp_, 128], [128 * DEp_, NT], [1, DEp_ - DE_]])
            nc.sync.dma_start(out=zdst, in_=zsrc)

        # ================= phase A: dense L1 =================
        hproj1 = big.tile([128, NT * D1], f32)
        for j in range(NT):
            xtile = dense.tile([128, 128], f32)
            nc.scalar.dma_start(out=xtile[:],
                                in_=xT_d[:, j * 128:(j + 1) * 128])
            ps = psum_tile([128, 2 * DE1 + D1])
            nc.tensor.matmul(out=ps[:], lhsT=xtile[:], rhs=W1cat[:],
                             start=True, stop=False)
            nc.tensor.matmul(out=ps[:], lhsT=ones1[:], rhs=b1cat[:],
                             start=False, stop=True)
            stg = dense.tile([128, 2 * DE1], f32, tag="stg1")
            nc.scalar.activation(out=stg[:], in_=ps[:, 0:2 * DE1], func=AF.Copy)
            nc.sync.dma_start(out=XL1_shard[j * 128:(j + 1) * 128, 0:DE1],
                              in_=stg[:, 0:DE1])
            nc.sync.dma_start(out=XR1[j * 128:(j + 1) * 128, 0:DE1],
                              in_=stg[:, DE1:2 * DE1])
            nc.scalar.activation(out=hproj1[:, j * D1:(j + 1) * D1],
                                 in_=ps[:, 2 * DE1:], func=AF.Copy)

        # ================= phase B: edge-attr mean =================
        ps0 = psum_tile([128, 1])
        PH0C = 8
        for t0_ in range(0, n_ph0, PH0C):
            ct = min(PH0C, n_ph0 - t0_)
            eat = dense.tile([128, PH0C * 128], f32, tag="ph0t", bufs=2)
            src_ap = bass.AP(tensor=ph0_d.tensor,
                             offset=ph0_d.offset + t0_ * 128 * 128,
                             ap=[[128, 128], [128 * 128, ct], [1, 128]])
            nc.sync.dma_start(
                out=eat[:, :ct * 128].rearrange("p (t c) -> p t c", c=128),
                in_=src_ap)
            for q_ in range(ct):
                t = t0_ + q_
                nc.tensor.matmul(out=ps0[:],
                                 lhsT=eat[:, q_ * 128:(q_ + 1) * 128],
                                 rhs=onesK[:],
                                 start=(t == 0), stop=(t == n_ph0 - 1))
        sae_sb = small.tile([128, 1], f32)
        nc.scalar.activation(out=sae_sb[:], in_=ps0[:], func=AF.Copy)
        nc.sync.dma_start(out=sae_l[:, :], in_=sae_sb[:])
        if no_cc:
            nc.sync.dma_start(out=sae_g[:, :], in_=sae_l[:, :])
        else:
            nc.gpsimd.collective_compute("AllReduce", OP.add, replica_groups=rg,
                                         ins=[sae_l[:].opt()],
                                         outs=[sae_g[:].opt()])
        # fold groups: [edim, n_grp] view of the 128-vector
        saeg = small.tile([edim, n_grp], f32)
        fold_src = bass.AP(tensor=sae_g.tensor, offset=sae_g[:].offset,
                           ap=[[1, edim], [edim, n_grp]])
        nc.sync.dma_start(out=saeg[:], in_=fold_src)
        vec16 = small.tile([edim, 1], f32)
        nc.vector.tensor_reduce(out=vec16[:, :], in_=saeg[:],
                                axis=mybir.AxisListType.X, op=OP.add)
        nc.vector.tensor_scalar_mul(out=vec16[:, :], in0=vec16[:, :],
                                    scalar1=1.0 / E)
        psc1 = psum_tile([1, DE1])
        nc.tensor.matmul(out=psc1[:], lhsT=vec16[:], rhs=We1[0:edim, :],
                         start=True, stop=True)
        c1sb = small.tile([1, DE1], f32)
        nc.scalar.activation(out=c1sb[:], in_=psc1[:], func=AF.Copy)
        nc.sync.dma_start(out=c1row[:, :], in_=c1sb[:])
        psc2 = psum_tile([1, DE2])
        nc.tensor.matmul(out=psc2[:], lhsT=vec16[:], rhs=We2[0:edim, :],
                         start=True, stop=True)
        c2sb = small.tile([1, DE2], f32)
        nc.scalar.activation(out=c2sb[:], in_=psc2[:], func=AF.Copy)
        nc.sync.dma_start(out=c2row[:, :], in_=c2sb[:])

        # ================= AllGather XL1 =================
        if no_cc:
            for c_ in range(ncores):
                nc.sync.dma_start(out=XL1_full[c_ * SHP:(c_ + 1) * SHP, :],
                                  in_=XL1_shard[:, :])
        else:
            nc.gpsimd.collective_compute("AllGather", OP.bypass,
                                         replica_groups=rg,
                                         ins=[XL1_shard[:].opt()],
                                         outs=[XL1_full[:].opt()])

        # ===== shared machinery =====
        def edge_phase(lname, XLfull, XR, Wet, DE, DEp, D, H, k_pos, acc,
                       Tw_, T_, half, srcd, dreld, eatd, evac_add):
            """One lo/hi pass of edge aggregation into acc [128, NT*DE]."""
            pw_map = {}
            n_blocks = int(math.ceil(T_ / BLK))
            xl_view = XLfull[half * HALF:(half + 1) * HALF, :]
            for b in range(n_blocks):
                t0 = b * BLK
                bt = min(BLK, T_ - t0)
                sd = blkp.tile([128, 18 * BLK], i16, tag="sd")
                nc.sync.dma_start(out=sd[:],
                                  in_=srcd[:, b * 18 * BLK:(b + 1) * 18 * BLK])
                soff = sd[:, 0:8 * BLK]
                doff = sd[:, 8 * BLK:16 * BLK]
                drel = sd[:, 16 * BLK:18 * BLK].bitcast(f32)
                eab = blkp.tile([edim + 1, BLK * 128], f32, tag="eab", bufs=1)
                nc.scalar.dma_start(out=eab[:, :bt * 128],
                                    in_=eatd[:, t0 * 128:(t0 + bt) * 128])
                xlb = blkp.tile([128, BLK * DEp1], f32, tag="xlb", bufs=1)
                xrb = blkp.tile([128, BLK * DEp1], f32, tag="xrb")
                gn = 128 if "gathers" in skip else bt * 128
                nc.gpsimd.dma_gather(
                    out_ap=xlb[:, :gn * DEp // 128].rearrange(
                        "p (t c) -> p t c", c=DEp),
                    in_ap=xl_view, idxs_ap=soff[:, :gn // 2 // 8],
                    num_idxs=gn, num_idxs_reg=gn,
                    elem_size=DEp, single_packet=False)
                nc.gpsimd.dma_gather(
                    out_ap=xrb[:, :gn * DEp // 128].rearrange(
                        "p (t c) -> p t c", c=DEp),
                    in_ap=XR[:, :], idxs_ap=doff[:, :gn // 2 // 8],
                    num_idxs=gn, num_idxs_reg=gn,
                    elem_size=DEp, single_packet=False)
                # one-hot P for the whole block
                Pb = blkp.tile([128, BLK * WIN], f32, tag="Pb", bufs=1)
                in0 = bass.AP(tensor=iota_f.tensor, offset=iota_f[:].offset,
                              ap=[list(iota_f[:].ap[0]), [0, bt], [1, WIN]])
                in1 = bass.AP(tensor=drel.tensor, offset=drel[:].offset,
                              ap=[list(drel[:].ap[0]), [1, bt], [0, WIN]])
                nbp = 1 if "pbuild" in skip else bt
                in0 = bass.AP(tensor=iota_f.tensor, offset=iota_f[:].offset,
                              ap=[list(iota_f[:].ap[0]), [0, nbp], [1, WIN]])
                in1 = bass.AP(tensor=drel.tensor, offset=drel[:].offset,
                              ap=[list(drel[:].ap[0]), [1, nbp], [0, WIN]])
                nc.vector.tensor_tensor(
                    out=Pb[:].rearrange("p (t w) -> p t w", w=WIN)[:, :nbp, :],
                    in0=in0, in1=in1, op=OP.is_equal)
                # e-matmuls in groups, then s = xl + xr + e (into xrb slots)
                n_g = int(math.ceil(bt / EGRP))
                for g in range(n_g):
                    gt0 = g * EGRP
                    gbt = min(EGRP, bt - gt0)
                    eps_ = psum_tile([128, EGRP * 128])
                    for q in range(1 if "emm" in skip else gbt):
                        nc.tensor.matmul(
                            out=eps_[:, q * 128:q * 128 + DE],
                            lhsT=eab[:, (gt0 + q) * 128:(gt0 + q + 1) * 128],
                            rhs=Wet[:], start=True, stop=True)
                    def sl3(tile_, width):
                        return bass.AP(
                            tensor=tile_.tensor,
                            offset=tile_[:].offset + gt0 * DEp,
                            ap=[list(tile_[:].ap[0]), [DEp, gbt], [1, width]])
                    if "adds" not in skip:
                        nc.vector.tensor_tensor(out=sl3(xrb, DE),
                                                in0=sl3(xrb, DE),
                                                in1=sl3(xlb, DE), op=OP.add)
                        eview = bass.AP(
                            tensor=eps_.tensor, offset=eps_[:].offset,
                            ap=[list(eps_[:].ap[0]), [128, gbt], [1, DE]])
                        nc.vector.tensor_tensor(out=sl3(xrb, DE),
                                                in0=sl3(xrb, DE),
                                                in1=eview, op=OP.add)
                    else:
                        eview = bass.AP(
                            tensor=eps_.tensor, offset=eps_[:].offset,
                            ap=[list(eps_[:].ap[0]), [128, 1], [1, DE]])
                        nc.vector.tensor_tensor(
                            out=sl3(xrb, DE) if gbt else sl3(xrb, DE),
                            in0=sl3(xrb, DE), in1=sl3(xlb, DE), op=OP.add)
                # abs-reduces (sign-grouped) -> z, exp, weighted features
                rq = small.tile([128, BLK * H], f32, tag="rq")
                nq = small.tile([128, BLK * H], f32, tag="nq")
                nc.vector.memset(rq[:], 0.0)
                nc.vector.memset(nq[:], 0.0)

                def red(dst_tile, h, c0, c1, neg):
                    if c1 <= c0:
                        return
                    src_ap = bass.AP(
                        tensor=xrb.tensor, offset=xrb[:].offset + h * hid + c0,
                        ap=[list(xrb[:].ap[0]), [DEp, bt], [1, c1 - c0]])
                    dst_ap = bass.AP(
                        tensor=dst_tile.tensor, offset=dst_tile[:].offset + h,
                        ap=[list(dst_tile[:].ap[0]), [H, bt]])
                    nc.vector.tensor_reduce(out=dst_ap, in_=src_ap,
                                            axis=mybir.AxisListType.X,
                                            op=OP.add, apply_absolute_value=True,
                                            negate=neg)
                if "reds" not in skip:
                    for h in range(H):
                        red(rq, h, 0, k_pos[h], False)
                        red(nq, h, k_pos[h], hid, True)
                nc.vector.tensor_add(out=rq[:, :bt * H], in0=rq[:, :bt * H],
                                     in1=nq[:, :bt * H])
                lin_ap = bass.AP(tensor=xrb.tensor, offset=xrb[:].offset + D,
                                 ap=[list(xrb[:].ap[0]), [DEp, bt], [1, H]])
                nc.vector.tensor_tensor(
                    out=rq[:, :bt * H].rearrange("p (t h) -> p t h", h=H),
                    in0=rq[:, :bt * H].rearrange("p (t h) -> p t h", h=H),
                    in1=lin_ap, op=OP.add)
                ex_out = bass.AP(tensor=xrb.tensor, offset=xrb[:].offset + D,
                                 ap=[list(xrb[:].ap[0]), [DEp, bt], [1, H]])
                nc.scalar.activation(out=ex_out,
                                     in_=rq[:, :bt * H].rearrange(
                                         "p (t h) -> p t h", h=H),
                                     func=AF.Exp, scale=0.4)
                exb = bass.AP(tensor=xrb.tensor, offset=xrb[:].offset + D,
                              ap=[list(xrb[:].ap[0]), [DEp, bt], [1, H], [0, hid]])
                w_out = bass.AP(tensor=xrb.tensor, offset=xrb[:].offset,
                                ap=[list(xrb[:].ap[0]), [DEp, bt], [hid, H], [1, hid]])
                xl_in = bass.AP(tensor=xlb.tensor, offset=xlb[:].offset,
                                ap=[list(xlb[:].ap[0]), [DEp, bt], [hid, H], [1, hid]])
                if "wmul" not in skip:
                    nc.vector.tensor_tensor(out=w_out, in0=xl_in, in1=exb,
                                            op=OP.mult)
                # scatter matmuls per tile (window PSUM persists across tiles)
                for q in range(bt):
                    t = t0 + q
                    w = t // Tw_
                    ti = t % Tw_
                    if ti == 0:
                        pw_map[w] = wpsum.tile([WIN, DE1], f32, space="PSUM",
                                               tag="pw", name=f"pw{w}")
                    pw = pw_map[w]
                    if "scatter" not in skip:
                        nc.tensor.matmul(out=pw[:, :DE],
                                         lhsT=Pb[:, q * WIN:(q + 1) * WIN],
                                         rhs=xrb[:, q * DEp:q * DEp + DE],
                                         start=(ti == 0), stop=(ti == Tw_ - 1))
                    else:
                        nc.tensor.matmul(out=pw[:, :DE],
                                         lhsT=Pb[:, 0:WIN],
                                         rhs=xrb[:, 0:DE],
                                         start=(ti == 0), stop=(ti == Tw_ - 1))
                    if ti == Tw_ - 1:
                        acc_sl = acc[(w % 2) * WIN:(w % 2) * WIN + WIN,
                                     (w // 2) * DE:(w // 2) * DE + DE]
                        if evac_add:
                            nc.vector.tensor_tensor(out=acc_sl, in0=acc_sl,
                                                    in1=pw[:, :DE], op=OP.add)
                        else:
                            nc.scalar.activation(out=acc_sl, in_=pw[:, :DE],
                                                 func=AF.Copy)
                        del pw_map[w]

        def selfloop_phase(XLsh, XRsh, crow_d, DE, DEp, D, H, k_pos, acc):
            cb = bcast_row(crow_d[:, :], DE)
            xls = big.tile([128, NT * DE1], f32, tag="xls")
            xrs = big.tile([128, NT * DE1], f32, tag="xrs")

            def ld(dst, src_dram):
                ap = bass.AP(tensor=src_dram.tensor, offset=src_dram[:].offset,
                             ap=[[DEp, 128], [128 * DEp, NT], [1, DE]])
                nc.sync.dma_start(
                    out=dst[:, :NT * DE].rearrange("p (j c) -> p j c", c=DE),
                    in_=ap)
            ld(xls, XLsh)
            ld(xrs, XRsh)
            sl = slice(0, NT * DE)
            nc.vector.tensor_add(out=xrs[:, sl], in0=xrs[:, sl], in1=xls[:, sl])
            cbb = bass.AP(tensor=cb.tensor, offset=cb[:].offset,
                          ap=[list(cb[:].ap[0]), [0, NT], [1, DE]])
            nc.vector.tensor_tensor(
                out=xrs[:, sl].rearrange("p (j c) -> p j c", c=DE),
                in0=xrs[:, sl].rearrange("p (j c) -> p j c", c=DE),
                in1=cbb, op=OP.add)
            rq = small.tile([128, NT * H], f32, tag="rqs")
            nq = small.tile([128, NT * H], f32, tag="nqs")
            nc.vector.memset(rq[:], 0.0)
            nc.vector.memset(nq[:], 0.0)

            def red(dst_tile, h, c0, c1, neg):
                if c1 <= c0:
                    return
                src_ap = bass.AP(
                    tensor=xrs.tensor, offset=xrs[:].offset + h * hid + c0,
                    ap=[list(xrs[:].ap[0]), [DE, NT], [1, c1 - c0]])
                dst_ap = bass.AP(
                    tensor=dst_tile.tensor, offset=dst_tile[:].offset + h,
                    ap=[list(dst_tile[:].ap[0]), [H, NT]])
                nc.vector.tensor_reduce(out=dst_ap, in_=src_ap,
                                        axis=mybir.AxisListType.X,
                                        op=OP.add, apply_absolute_value=True,
                                        negate=neg)
            for h in range(H):
                red(rq, h, 0, k_pos[h], False)
                red(nq, h, k_pos[h], hid, True)
            nc.vector.tensor_add(out=rq[:], in0=rq[:], in1=nq[:])
            lin_ap = bass.AP(tensor=xrs.tensor, offset=xrs[:].offset + D,
                             ap=[list(xrs[:].ap[0]), [DE, NT], [1, H]])
            nc.vector.tensor_tensor(
                out=rq[:].rearrange("p (j h) -> p j h", h=H),
                in0=rq[:].rearrange("p (j h) -> p j h", h=H),
                in1=lin_ap, op=OP.add)
            ex_out = bass.AP(tensor=xrs.tensor, offset=xrs[:].offset + D,
                             ap=[list(xrs[:].ap[0]), [DE, NT], [1, H]])
            nc.scalar.activation(out=ex_out,
                                 in_=rq[:].rearrange("p (j h) -> p j h", h=H),
                                 func=AF.Exp, scale=0.4)
            exb = bass.AP(tensor=xrs.tensor, offset=xrs[:].offset + D,
                          ap=[list(xrs[:].ap[0]), [DE, NT], [1, H], [0, hid]])
            w_out = bass.AP(tensor=xrs.tensor, offset=xrs[:].offset,
                            ap=[list(xrs[:].ap[0]), [DE, NT], [hid, H], [1, hid]])
            xl_in = bass.AP(tensor=xls.tensor, offset=xls[:].offset,
                            ap=[list(xls[:].ap[0]), [DE, NT], [hid, H], [1, hid]])
            nc.vector.tensor_tensor(out=w_out, in0=xl_in, in1=exb, op=OP.mult)
            nc.vector.tensor_add(out=acc[:, :NT * DE], in0=acc[:, :NT * DE],
                                 in1=xrs[:, :NT * DE])

        def finalize_phase(acc, biasb, bnsb, hproj, stl, stg, gsr, shr,
                           DE, D, H, lname):
            """acc -> y (in ybuf) -> BN+skip+ELU -> h (returned)."""
            den = small.tile([128, NT * H], f32, tag="den")
            den_src = bass.AP(tensor=acc.tensor, offset=acc[:].offset + D,
                              ap=[list(acc[:].ap[0]), [DE, NT], [1, H]])
            nc.vector.tensor_scalar_add(
                out=den[:].rearrange("p (j h) -> p j h", h=H),
                in0=den_src, scalar1=1e-16)
            rec = small.tile([128, NT * H], f32, tag="rec")
            nc.vector.reciprocal(out=rec[:], in_=den[:])
            ybuf = big.tile([128, NT * D1], f32, tag="ybuf", name="ybuf")
            recb = bass.AP(tensor=rec.tensor, offset=rec[:].offset,
                           ap=[list(rec[:].ap[0]), [H, NT], [1, H], [0, hid]])
            num_in = bass.AP(tensor=acc.tensor, offset=acc[:].offset,
                             ap=[list(acc[:].ap[0]), [DE, NT], [hid, H], [1, hid]])
            y_out = bass.AP(tensor=ybuf.tensor, offset=ybuf[:].offset,
                            ap=[list(ybuf[:].ap[0]), [D, NT], [hid, H], [1, hid]])
            nc.vector.tensor_tensor(out=y_out, in0=num_in, in1=recb, op=OP.mult)
            bb = bass.AP(tensor=biasb.tensor, offset=biasb[:].offset,
                         ap=[list(biasb[:].ap[0]), [0, NT], [1, D]])
            yv = ybuf[:, :NT * D].rearrange("p (j c) -> p j c", c=D)
            nc.vector.tensor_tensor(out=yv, in0=yv, in1=bb, op=OP.add)
            if dbg and lname == "L1":
                nc.sync.dma_start(out=dbg_y1[:, :NT * D], in_=ybuf[:, :NT * D])
            # mask pads to 0 (also needed for stats)
            mb = bass.AP(tensor=maskB.tensor, offset=maskB[:].offset,
                         ap=[list(maskB[:].ap[0]), [1, NT], [0, D]])
            nc.vector.tensor_tensor(out=yv, in0=yv, in1=mb, op=OP.mult)
            # stats
            sq = big.tile([128, NT * D1], f32, tag="xls", name="sqb")
            nc.scalar.activation(out=sq[:, :NT * D], in_=ybuf[:, :NT * D],
                                 func=AF.Square)
            ysum = small.tile([128, D1], f32, tag="ysum")
            s_in = bass.AP(tensor=ybuf.tensor, offset=ybuf[:].offset,
                           ap=[list(ybuf[:].ap[0]), [1, D], [D, NT]])
            nc.vector.tensor_reduce(out=ysum[:, :D], in_=s_in,
                                    axis=mybir.AxisListType.X, op=OP.add)
            qsum = small.tile([128, D1], f32, tag="qsum")
            q_in = bass.AP(tensor=sq.tensor, offset=sq[:].offset,
                           ap=[list(sq[:].ap[0]), [1, D], [D, NT]])
            nc.vector.tensor_reduce(out=qsum[:, :D], in_=q_in,
                                    axis=mybir.AxisListType.X, op=OP.add)
            pst = psum_tile([D1, 2])
            nc.tensor.matmul(out=pst[:D, 0:1], lhsT=ysum[:, :D], rhs=onesK[:],
                             start=True, stop=True)
            nc.tensor.matmul(out=pst[:D, 1:2], lhsT=qsum[:, :D], rhs=onesK[:],
                             start=True, stop=True)
            stsb = small.tile([D1, 2], f32, tag="stsb")
            nc.scalar.activation(out=stsb[:D, :], in_=pst[:D, :], func=AF.Copy)
            st_view = bass.AP(tensor=stl.tensor, offset=stl[:].offset,
                              ap=[[2, D], [1, 2]])
            nc.sync.dma_start(out=st_view, in_=stsb[:D, :])
            if no_cc:
                nc.sync.dma_start(out=stg[:, :], in_=stl[:, :])
            else:
                nc.gpsimd.collective_compute("AllReduce", OP.add,
                                             replica_groups=rg,
                                             ins=[stl[:].opt()],
                                             outs=[stg[:].opt()])
            if dbg and lname == "L1":
                stg_dump = small.tile([D1, 2], f32, tag="stgdump")
                stg_view_d = bass.AP(tensor=stg.tensor, offset=stg[:].offset,
                                     ap=[[2, D], [1, 2]])
                nc.sync.dma_start(out=stg_dump[:D, :], in_=stg_view_d)
                nc.sync.dma_start(out=dbg_st1[:D, :], in_=stg_dump[:D, :])
            stg_sb = small.tile([D1, 2], f32, tag="stgsb")
            stg_view = bass.AP(tensor=stg.tensor, offset=stg[:].offset,
                               ap=[[2, D], [1, 2]])
            nc.sync.dma_start(out=stg_sb[:D, :], in_=stg_view)
            mu = small.tile([D1, 1], f32, tag="mu")
            nc.vector.tensor_scalar_mul(out=mu[:D, :], in0=stg_sb[:D, 0:1],
                                        scalar1=1.0 / N)
            var = small.tile([D1, 1], f32, tag="var")
            nc.vector.tensor_scalar_mul(out=var[:D, :], in0=stg_sb[:D, 1:2],
                                        scalar1=1.0 / N)
            musq = small.tile([D1, 1], f32, tag="musq")
            nc.vector.tensor_mul(out=musq[:D, :], in0=mu[:D, :], in1=mu[:D, :])
            nc.vector.tensor_sub(out=var[:D, :], in0=var[:D, :], in1=musq[:D, :])
            # rstd = exp(-0.5*ln(var+eps_c)) with per-channel scaled eps
            nc.vector.tensor_add(out=var[:D, :], in0=var[:D, :],
                                 in1=bnsb[:D, 2:3])
            nc.scalar.activation(out=var[:D, :], in_=var[:D, :], func=AF.Ln)
            nc.scalar.activation(out=var[:D, :], in_=var[:D, :], func=AF.Exp,
                                 scale=-0.5)
            gsc = small.tile([D1, 1], f32, tag="gsc")
            nc.vector.tensor_mul(out=gsc[:D, :], in0=bnsb[:D, 0:1], in1=var[:D, :])
            shf = small.tile([D1, 1], f32, tag="shf")
            nc.vector.tensor_mul(out=shf[:D, :], in0=mu[:D, :], in1=gsc[:D, :])
            nc.vector.tensor_sub(out=shf[:D, :], in0=bnsb[:D, 1:2], in1=shf[:D, :])
            # roundtrip to DRAM rows for broadcast
            nc.sync.dma_start(out=bass.AP(tensor=gsr.tensor, offset=gsr[:].offset,
                                          ap=[[1, D], [1, 1]]),
                              in_=gsc[:D, :])
            nc.sync.dma_start(out=bass.AP(tensor=shr.tensor, offset=shr[:].offset,
                                          ap=[[1, D], [1, 1]]),
                              in_=shf[:D, :])
            gscb = bcast_row(gsr[:, :], D)
            shfb = bcast_row(shr[:, :], D)
            # v = y*gsc + shf + hproj ; h = relu(v) + exp(min(v,0)) - 1
            gb = bass.AP(tensor=gscb.tensor, offset=gscb[:].offset,
                         ap=[list(gscb[:].ap[0]), [0, NT], [1, D]])
            sb_ = bass.AP(tensor=shfb.tensor, offset=shfb[:].offset,
                          ap=[list(shfb[:].ap[0]), [0, NT], [1, D]])
            nc.vector.tensor_tensor(out=yv, in0=yv, in1=gb, op=OP.mult)
            nc.vector.tensor_tensor(out=yv, in0=yv, in1=sb_, op=OP.add)
            nc.vector.tensor_add(out=ybuf[:, :NT * D], in0=ybuf[:, :NT * D],
                                 in1=hproj[:, :NT * D])
            hbuf = big.tile([128, NT * D1], f32, tag="hbuf", name="hbuf")
            nc.vector.tensor_scalar_min(out=hbuf[:, :NT * D],
                                        in0=ybuf[:, :NT * D], scalar1=0.0)
            nc.scalar.activation(out=hbuf[:, :NT * D], in_=hbuf[:, :NT * D],
                                 func=AF.Exp)
            nc.scalar.activation(out=ybuf[:, :NT * D], in_=ybuf[:, :NT * D],
                                 func=AF.Relu)
            nc.vector.tensor_add(out=hbuf[:, :NT * D], in0=hbuf[:, :NT * D],
                                 in1=ybuf[:, :NT * D])
            nc.vector.tensor_scalar_add(out=hbuf[:, :NT * D],
                                        in0=hbuf[:, :NT * D], scalar1=-1.0)
            return hbuf

        # ================= L1 edge + selfloop + finalize =================
        acc1 = big.tile([128, NT * DE1], f32, tag="acc1")
        edge_phase("L1", XL1_full, XR1, We1, DE1, DEp1, D1, heads,
                   meta["k_pos1"], acc1, TwA, TA, 0,
                   sdA_d, drelA_d, eaTA_d, evac_add=False)
        edge_phase("L1", XL1_full, XR1, We1, DE1, DEp1, D1, heads,
                   meta["k_pos1"], acc1, TwB, TB, 1,
                   sdB_d, drelB_d, eaTB_d, evac_add=True)
        selfloop_phase(XL1_shard, XR1, c1row, DE1, DEp1, D1, heads,
                       meta["k_pos1"], acc1)
        h1 = finalize_phase(acc1, bias1b, bn1sb, hproj1, st1_l, st1_g,
                            gs1row, sh1row, DE1, D1, heads, "L1")
        if dbg:
            nc.sync.dma_start(out=dbg_xl1[:, :], in_=XL1_shard[:, :])
            nc.sync.dma_start(out=dbg_xr1[:, :], in_=XR1[:, :])
            nc.sync.dma_start(out=dbg_acc1[:, :], in_=acc1[:, :])
            nc.sync.dma_start(out=dbg_h1[:, :], in_=h1[:, :NT * D1])
